# revision 36
# baseline (speedup 1.0000x reference)
"""Bayesian linear layer (reparameterized per-sample weights) on 8 trn2 NeuronCores.

y[b,o] = sum_i x[b,i] * (mu[o,i] + softplus(rho[o,i]) * eps_w[b,o,i])
         + bias_mu[o] + softplus(bias_rho[o]) * eps_b[b,o]

Sharding: data-parallel over batch. 8 cores x 32 samples. mu/rho replicated.

Natural-layout pipeline (no PE transposes of eps; the 134 MB eps_w shard per
core at ~360 GB/s HBM is the ~390 us roofline):
  1. SWDGE cast-DMA: eps_w[b] fp32 HBM -> bf16 SBUF, natural tiles
     [o=128p, c=8, i=1024] (contiguous 4KB runs, full BW, half SBUF write)
  2. DVE TT1 (bf16 2x): t1 = eps (*) sigma_nat          one instr FD=8192
  3. DVE TT2 (bf16 2x): t2 = t1 (*) Xb (x[b,:] bcast)   one instr FD=8192
  4. reduce over i per o-chunk: ScalarE ACTIVATE(accum_out) for N_ACT chunks,
     DVE tensor_reduce for the rest (engine balance)
  5. y2 columns accumulate in [128, 8, BL]; final 8 PE transposes -> + C
  C[b,o] = x@mu.T (bf16 PE matmul) + bias_mu + softplus(bias_rho)*eps_b.

Issue order tuned for startup: sigma + first eps tiles go first; the mu/ymu/C
setup block is issued after sample 3 so it runs on otherwise-idle units
(sync-queue DMA, PE) without delaying the hot pipeline.
"""

import numpy as np

import concourse.bass as bass
from concourse import bacc
import concourse.mybir as mybir
import concourse.tile as tile
from concourse.bass import ts
from concourse.bass_utils import run_bass_kernel_spmd
from concourse.masks import make_identity

FP32 = mybir.dt.float32
BF16 = mybir.dt.bfloat16
AF = mybir.ActivationFunctionType

F = 1024          # feature dim (in == out)
N_CORES = 8
NCH = F // 128    # 8 o-chunks of 128

# chunks 0..N_ACT-1 reduce on ScalarE (ACTIVATE accum_out), rest on DVE
N_ACT = 7
# sample index after which the mu/ymu/C setup block is issued
MU_AFTER = 3


def build_nc(BL: int, eps_bufs=3, t_bufs=2) -> bass.Bass:
    nc = bacc.Bacc(None, target_bir_lowering=False)

    x_d = nc.declare_dram_parameter("x", [BL, F], FP32, isOutput=False)
    mu_d = nc.declare_dram_parameter("weight_mu", [F, F], FP32, isOutput=False)
    rho_d = nc.declare_dram_parameter("weight_rho", [F, F], FP32, isOutput=False)
    bmu_d = nc.declare_dram_parameter("bias_mu", [F], FP32, isOutput=False)
    brho_d = nc.declare_dram_parameter("bias_rho", [F], FP32, isOutput=False)
    epsw_d = nc.declare_dram_parameter("eps_w", [BL, F, F], FP32, isOutput=False)
    epsb_d = nc.declare_dram_parameter("eps_b", [BL, F], FP32, isOutput=False)
    y_d = nc.declare_dram_parameter("y", [BL, F], FP32, isOutput=True)

    # [b, (c p), i] -> [b, p, c, i]: partition p = o within chunk c (natural)
    epsw_t = epsw_d[:].rearrange("b (c p) i -> b p c i", p=128)
    mu_t = mu_d[:].rearrange("(c p) i -> p c i", p=128)
    rho_t = rho_d[:].rearrange("(c p) i -> p c i", p=128)

    with tile.TileContext(nc) as tc:
        with (
            tc.tile_pool(name="persist", bufs=1) as persist,
            tc.tile_pool(name="setup", bufs=4) as setp,
            tc.tile_pool(name="mus", bufs=2) as musp,
            tc.tile_pool(name="xr", bufs=2) as xrp,
            tc.tile_pool(name="eps", bufs=eps_bufs) as epsp,
            tc.tile_pool(name="t1", bufs=t_bufs) as t1p,
            tc.tile_pool(name="t2", bufs=3) as t2p,
            tc.tile_pool(name="xb", bufs=2) as xbp,
            tc.tile_pool(name="junk", bufs=2) as junkp,
            tc.tile_pool(name="pt", bufs=1, space="PSUM") as ptp,
            tc.tile_pool(name="pty", bufs=1, space="PSUM") as ptyp,
            tc.tile_pool(name="ymu", bufs=1, space="PSUM") as ymup,
            tc.tile_pool(name="xps", bufs=1, space="PSUM") as xpsp,
        ):
            # ---------------- minimal setup for the hot loop ----------------
            ident = persist.tile([128, 128], FP32)
            make_identity(nc, ident)
            ones_row = persist.tile([1, 128], FP32)
            nc.vector.memset(ones_row, 1.0)

            # first two samples' x rows: issued before the rho DMAs so the
            # sync queue delivers them immediately (xb feeds TT2 early)
            xrow_pre = []
            for b in range(min(2, BL)):
                xrow_b = xrp.tile([1, F], FP32, tag="xr")
                nc.sync.dma_start(out=xrow_b, in_=x_d[b : b + 1, :])
                xrow_pre.append(xrow_b)

            # sigma natural bf16 = softplus(rho); batch Exp then Ln in halves
            # of 4 chunks to limit ACT table switches (4 loads, not 16)
            sig = persist.tile([128, NCH, F], BF16)
            for half in range(2):
                cs = range(half * 4, half * 4 + 4)
                sp_cs = []
                for c in cs:
                    rho_c = setp.tile([128, F], FP32, tag="s")
                    nc.sync.dma_start(out=rho_c, in_=rho_t[:, c, :])
                    # softplus(x) = ln(1 + exp(x)); rho <= ~0 so no overflow;
                    # exp in place
                    nc.scalar.activation(out=rho_c, in_=rho_c, func=AF.Exp)
                    sp_cs.append(rho_c)
                for c, sp_c in zip(cs, sp_cs):
                    nc.scalar.activation(
                        out=sig[:, c, :], in_=sp_c, func=AF.Ln, bias=1.0
                    )

            # bias terms: broadcast-DMA rows, softplus while Exp/Ln tables hot
            bmu_b = persist.tile([BL, F], FP32)
            nc.gpsimd.dma_start(
                out=bmu_b,
                in_=bass.AP(tensor=bmu_d, offset=0, ap=[[0, BL], [1, F]]),
            )
            sb_b = persist.tile([BL, F], FP32)
            nc.gpsimd.dma_start(
                out=sb_b,
                in_=bass.AP(tensor=brho_d, offset=0, ap=[[0, BL], [1, F]]),
            )
            epsb_s = persist.tile([BL, F], FP32)
            nc.sync.dma_start(out=epsb_s, in_=epsb_d[:])
            nc.scalar.activation(out=sb_b, in_=sb_b, func=AF.Exp)
            nc.scalar.activation(out=sb_b, in_=sb_b, func=AF.Ln, bias=1.0)

            # y2 accumulator columns [128p(o in c), c, b]
            y2cols = persist.tile([128, NCH, BL], FP32)

            # ---------------- main loop over samples ----------------
            def reduce_chunk(b, c, on_act, t2):
                if on_act:
                    junk = junkp.tile([128, F], BF16, tag="junk")
                    nc.scalar.activation(
                        out=junk,
                        in_=t2[:, c, :],
                        func=AF.Copy,
                        accum_out=y2cols[:, c, b : b + 1],
                    )
                else:
                    nc.vector.tensor_reduce(
                        out=y2cols[:, c, b : b + 1],
                        in_=t2[:, c, :],
                        axis=mybir.AxisListType.X,
                        op=mybir.AluOpType.add,
                    )

            def make_xb(b):
                # x[b,:] broadcast to all partitions: tiny DMA of the row to
                # partition 0 (HWDGE queue - must not sit behind eps DMAs on
                # the SWDGE queue), then PE outer product ones.T @ xrow_b
                if b < len(xrow_pre):
                    xrow_b = xrow_pre[b]
                else:
                    xrow_b = xrp.tile([1, F], FP32, tag="xr")
                    nc.sync.dma_start(out=xrow_b, in_=x_d[b : b + 1, :])
                xps = xpsp.tile([128, F], FP32, tag="xps")
                for h in range(2):
                    nc.tensor.matmul(
                        out=xps[:, ts(h, 512)],
                        lhsT=ones_row,
                        rhs=xrow_b[:, ts(h, 512)],
                        start=True,
                        stop=True,
                    )
                xb = xbp.tile([128, F], BF16)
                nc.scalar.copy(out=xb, in_=xps)
                return xb

            def sample0_body(b):
                # chunk-granular first sample: starts as soon as the first
                # sigma chunk and first eps chunk land
                xb = make_xb(b)
                eb = epsp.tile([128, NCH, F], BF16, tag="eps")
                for c in range(NCH):
                    nc.gpsimd.dma_start(
                        out=eb[:, c, :], in_=epsw_t[b][:, c, :]
                    )
                t2 = t2p.tile([128, NCH, F], BF16, tag="t2")
                n_act = 7 if b % 2 == 0 else 6
                for c in range(NCH):
                    if c % 4 == 0:
                        t1h = t1p.tile([128, 4, F], BF16, tag="t1")
                    nc.vector.tensor_mul(t1h[:, c % 4, :], eb[:, c, :], sig[:, c, :])
                    nc.vector.tensor_mul(t2[:, c, :], t1h[:, c % 4, :], xb)
                    reduce_chunk(b, c, c < n_act, t2)

            def sample_body(b):
                xb = make_xb(b)

                # eps in two half-DMAs so the first TT can start sooner
                eb = epsp.tile([128, NCH, F], BF16, tag="eps")
                nc.gpsimd.dma_start(out=eb[:, :4, :], in_=epsw_t[b][:, :4, :])
                nc.gpsimd.dma_start(out=eb[:, 4:, :], in_=epsw_t[b][:, 4:, :])

                t2 = t2p.tile([128, NCH, F], BF16, tag="t2")
                xb_bc = xb[:].unsqueeze(1).broadcast_to([128, 4, F])
                n_act = 7 if b % 2 == 0 else 6
                for h in range(2):
                    hs = slice(h * 4, h * 4 + 4)
                    t1 = t1p.tile([128, 4, F], BF16, tag="t1")
                    nc.vector.tensor_mul(t1, eb[:, hs, :], sig[:, hs, :])
                    nc.vector.tensor_mul(t2[:, hs, :], t1, xb_bc)
                    for c in range(h * 4, h * 4 + 4):
                        reduce_chunk(b, c, c < n_act, t2)

            # mu/ymu setup pieces, spread one chunk per sample through the
            # early main loop so they never stall the hot pipeline
            muT = persist.tile([128, NCH, F], BF16)
            x_nat = persist.tile([BL, F], FP32)
            xT = persist.tile([128, NCH, BL], BF16)
            ymu_ps = []

            def do_mu_chunk(c):
                # muT bf16 [128p(i in k), k, o] via fp32 PE transposes
                mu_c = musp.tile([128, F], FP32, tag="mu")
                nc.sync.dma_start(out=mu_c, in_=mu_t[:, c, :])
                pt_c = ptp.tile([128, F], FP32, tag="pt")
                for k in range(NCH):
                    nc.tensor.transpose(
                        out=pt_c[:, ts(k, 128)],
                        in_=mu_c[:, ts(k, 128)],
                        identity=ident,
                    )
                # pt_c holds [i in k (part), (k, o in c)]; scatter to muT
                nc.scalar.copy(
                    out=muT[:, :, ts(c, 128)],
                    in_=pt_c[:].rearrange("p (k j) -> p k j", k=NCH),
                )

            def do_ymu():
                nc.sync.dma_start(out=x_nat, in_=x_d[:])
                for k in range(NCH):
                    ptx = ptp.tile([128, F], FP32, tag="pt")
                    nc.tensor.transpose(
                        out=ptx[:, :BL],
                        in_=x_nat[:, ts(k, 128)],
                        identity=ident[:BL, :BL],
                    )
                    nc.scalar.copy(out=xT[:, k, :], in_=ptx[:, :BL])
                # y_mu[b, o] = sum_i x[b,i] mu[o,i]  (bf16 matmul)
                for h in range(2):
                    yp = ymup.tile([BL, 512], FP32, tag=f"ymu{h}")
                    for k in range(NCH):
                        nc.tensor.matmul(
                            out=yp,
                            lhsT=xT[:, k, :],
                            rhs=muT[:, k, ts(h, 512)],
                            start=(k == 0),
                            stop=(k == NCH - 1),
                        )
                    ymu_ps.append(yp)

            mu_done = 0
            sample0_body(0)
            for b in range(1, BL):
                sample_body(b)
                if b >= 3 and mu_done < NCH:
                    do_mu_chunk(mu_done)
                    mu_done += 1
                if b == 12:
                    do_ymu()
            while mu_done < NCH:
                do_mu_chunk(mu_done)
                mu_done += 1
            if not ymu_ps:
                do_ymu()

            # -------------- C assembly + final: y = C + y2^T --------------
            C = persist.tile([BL, F], FP32)
            nc.vector.tensor_mul(C, sb_b, epsb_s)
            nc.vector.tensor_add(C, C, bmu_b)
            for h in range(2):
                nc.vector.tensor_add(C[:, ts(h, 512)], C[:, ts(h, 512)], ymu_ps[h])
            # all 8 y2 transposes into one PSUM tile, one add, one store
            pty = ptyp.tile([BL, F], FP32, tag="pty")
            for c in range(NCH):
                nc.tensor.transpose(
                    out=pty[:, ts(c, 128)], in_=y2cols[:, c, :], identity=ident
                )
            nc.vector.tensor_add(C, C, pty)
            nc.sync.dma_start(out=y_d[:], in_=C)

    nc.compile()
    return nc


_NC_CACHE: dict[int, bass.Bass] = {}


def _get_nc(BL: int) -> bass.Bass:
    if BL not in _NC_CACHE:
        _NC_CACHE[BL] = build_nc(BL)
    return _NC_CACHE[BL]


def kernel(x, weight_mu, weight_rho, bias_mu, bias_rho, eps_w, eps_b):
    B = x.shape[0]
    BL = B // N_CORES
    nc = _get_nc(BL)

    x = np.ascontiguousarray(np.asarray(x, dtype=np.float32))
    weight_mu = np.ascontiguousarray(np.asarray(weight_mu, dtype=np.float32))
    weight_rho = np.ascontiguousarray(np.asarray(weight_rho, dtype=np.float32))
    bias_mu = np.ascontiguousarray(np.asarray(bias_mu, dtype=np.float32))
    bias_rho = np.ascontiguousarray(np.asarray(bias_rho, dtype=np.float32))
    eps_w = np.ascontiguousarray(np.asarray(eps_w, dtype=np.float32))
    eps_b = np.ascontiguousarray(np.asarray(eps_b, dtype=np.float32))

    in_maps = []
    for i in range(N_CORES):
        sl = slice(i * BL, (i + 1) * BL)
        in_maps.append(
            {
                "x": x[sl],
                "weight_mu": weight_mu,
                "weight_rho": weight_rho,
                "bias_mu": bias_mu,
                "bias_rho": bias_rho,
                "eps_w": eps_w[sl],
                "eps_b": eps_b[sl],
            }
        )

    res = run_bass_kernel_spmd(nc, in_maps, core_ids=list(range(N_CORES)))
    return np.concatenate([r["y"] for r in res.results], axis=0)


# revision 39
# speedup vs baseline: 1.0716x; 1.0716x over previous
"""Bayesian linear layer (reparameterized per-sample weights) on 8 trn2 NeuronCores.

y[b,o] = sum_i x[b,i] * (mu[o,i] + softplus(rho[o,i]) * eps_w[b,o,i])
         + bias_mu[o] + softplus(bias_rho[o]) * eps_b[b,o]

Sharding: data-parallel over batch. 8 cores x 32 samples. mu/rho replicated.

Natural-layout pipeline (no PE transposes of eps; the 134 MB eps_w shard per
core at ~360 GB/s HBM is the ~390 us roofline):
  1. SWDGE cast-DMA: eps_w[b] fp32 HBM -> bf16 SBUF, natural tiles
     [o=128p, c=8, i=1024] (contiguous 4KB runs, full BW, half SBUF write)
  2. DVE TT1 (bf16 2x): t1 = eps (*) sigma_nat          one instr FD=8192
  3. DVE TT2 (bf16 2x): t2 = t1 (*) Xb (x[b,:] bcast)   one instr FD=8192
  4. reduce over i per o-chunk: ScalarE ACTIVATE(accum_out) for N_ACT chunks,
     DVE tensor_reduce for the rest (engine balance)
  5. y2 columns accumulate in [128, 8, BL]; final 8 PE transposes -> + C
  C[b,o] = x@mu.T (bf16 PE matmul) + bias_mu + softplus(bias_rho)*eps_b.

Issue order tuned for startup: sigma + first eps tiles go first; the mu/ymu/C
setup block is issued after sample 3 so it runs on otherwise-idle units
(sync-queue DMA, PE) without delaying the hot pipeline.
"""

import numpy as np

import concourse.bass as bass
from concourse import bacc
import concourse.mybir as mybir
import concourse.tile as tile
from concourse.bass import ts
from concourse.bass_utils import run_bass_kernel_spmd
from concourse.masks import make_identity

FP32 = mybir.dt.float32
BF16 = mybir.dt.bfloat16
AF = mybir.ActivationFunctionType

F = 1024          # feature dim (in == out)
N_CORES = 8
NCH = F // 128    # 8 o-chunks of 128

# chunks 0..N_ACT-1 reduce on ScalarE (ACTIVATE accum_out), rest on DVE
N_ACT = 7
# sample index after which the mu/ymu/C setup block is issued
MU_AFTER = 3


def build_nc(BL: int, eps_bufs=3, t_bufs=2) -> bass.Bass:
    nc = bacc.Bacc(None, target_bir_lowering=False)

    x_d = nc.declare_dram_parameter("x", [BL, F], FP32, isOutput=False)
    mu_d = nc.declare_dram_parameter("weight_mu", [F, F], FP32, isOutput=False)
    rho_d = nc.declare_dram_parameter("weight_rho", [F, F], FP32, isOutput=False)
    bmu_d = nc.declare_dram_parameter("bias_mu", [F], FP32, isOutput=False)
    brho_d = nc.declare_dram_parameter("bias_rho", [F], FP32, isOutput=False)
    epsw_d = nc.declare_dram_parameter("eps_w", [BL, F, F], FP32, isOutput=False)
    epsb_d = nc.declare_dram_parameter("eps_b", [BL, F], FP32, isOutput=False)
    y_d = nc.declare_dram_parameter("y", [BL, F], FP32, isOutput=True)

    # [b, (c p), i] -> [b, p, c, i]: partition p = o within chunk c (natural)
    epsw_t = epsw_d[:].rearrange("b (c p) i -> b p c i", p=128)
    mu_t = mu_d[:].rearrange("(c p) i -> p c i", p=128)
    rho_t = rho_d[:].rearrange("(c p) i -> p c i", p=128)

    with tile.TileContext(nc) as tc:
        with (
            tc.tile_pool(name="persist", bufs=1) as persist,
            tc.tile_pool(name="setup", bufs=4) as setp,
            tc.tile_pool(name="mus", bufs=2) as musp,
            tc.tile_pool(name="xr", bufs=2) as xrp,
            tc.tile_pool(name="eps", bufs=eps_bufs) as epsp,
            tc.tile_pool(name="t1", bufs=t_bufs) as t1p,
            tc.tile_pool(name="t2", bufs=t_bufs) as t2p,
            tc.tile_pool(name="xb", bufs=2) as xbp,
            tc.tile_pool(name="junk", bufs=2) as junkp,
            tc.tile_pool(name="pt", bufs=1, space="PSUM") as ptp,
            tc.tile_pool(name="pty", bufs=1, space="PSUM") as ptyp,
            tc.tile_pool(name="ymu", bufs=1, space="PSUM") as ymup,
            tc.tile_pool(name="xps", bufs=1, space="PSUM") as xpsp,
        ):
            # ---------------- minimal setup for the hot loop ----------------
            ident = persist.tile([128, 128], FP32)
            make_identity(nc, ident)
            ones_row = persist.tile([1, 128], FP32)
            nc.vector.memset(ones_row, 1.0)

            # first two samples' x rows: issued before the rho DMAs so the
            # sync queue delivers them immediately (xb feeds TT2 early)
            xrow_pre = []
            for b in range(min(2, BL)):
                xrow_b = xrp.tile([1, F], FP32, tag="xr")
                nc.sync.dma_start(out=xrow_b, in_=x_d[b : b + 1, :])
                xrow_pre.append(xrow_b)

            # sigma natural bf16 = softplus(rho); batch Exp then Ln in halves
            # of 4 chunks to limit ACT table switches (4 loads, not 16)
            sig = persist.tile([128, NCH, F], BF16)
            for half in range(2):
                cs = range(half * 4, half * 4 + 4)
                sp_cs = []
                for c in cs:
                    rho_c = setp.tile([128, F], FP32, tag="s")
                    nc.sync.dma_start(out=rho_c, in_=rho_t[:, c, :])
                    # softplus(x) = ln(1 + exp(x)); rho <= ~0 so no overflow;
                    # exp in place
                    nc.scalar.activation(out=rho_c, in_=rho_c, func=AF.Exp)
                    sp_cs.append(rho_c)
                for c, sp_c in zip(cs, sp_cs):
                    nc.scalar.activation(
                        out=sig[:, c, :], in_=sp_c, func=AF.Ln, bias=1.0
                    )

            # bias terms: broadcast-DMA rows, softplus while Exp/Ln tables hot
            bmu_b = persist.tile([BL, F], FP32)
            nc.gpsimd.dma_start(
                out=bmu_b,
                in_=bass.AP(tensor=bmu_d, offset=0, ap=[[0, BL], [1, F]]),
            )
            sb_b = persist.tile([BL, F], FP32)
            nc.gpsimd.dma_start(
                out=sb_b,
                in_=bass.AP(tensor=brho_d, offset=0, ap=[[0, BL], [1, F]]),
            )
            epsb_s = persist.tile([BL, F], FP32)
            nc.sync.dma_start(out=epsb_s, in_=epsb_d[:])
            nc.scalar.activation(out=sb_b, in_=sb_b, func=AF.Exp)
            nc.scalar.activation(out=sb_b, in_=sb_b, func=AF.Ln, bias=1.0)

            # y2 accumulator columns [128p(o in c), c, b]
            y2cols = persist.tile([128, NCH, BL], FP32)

            # ---------------- main loop over samples ----------------
            def reduce_chunk(b, c, on_act, t2):
                if on_act:
                    junk = junkp.tile([128, F], BF16, tag="junk")
                    nc.scalar.activation(
                        out=junk,
                        in_=t2[:, c, :],
                        func=AF.Copy,
                        accum_out=y2cols[:, c, b : b + 1],
                    )
                else:
                    nc.vector.tensor_reduce(
                        out=y2cols[:, c, b : b + 1],
                        in_=t2[:, c, :],
                        axis=mybir.AxisListType.X,
                        op=mybir.AluOpType.add,
                    )

            def make_xb(b):
                # x[b,:] broadcast to all partitions: tiny DMA of the row to
                # partition 0 (HWDGE queue - must not sit behind eps DMAs on
                # the SWDGE queue), then PE outer product ones.T @ xrow_b
                if b < len(xrow_pre):
                    xrow_b = xrow_pre[b]
                else:
                    xrow_b = xrp.tile([1, F], FP32, tag="xr")
                    nc.sync.dma_start(out=xrow_b, in_=x_d[b : b + 1, :])
                xps = xpsp.tile([128, F], FP32, tag="xps")
                for h in range(2):
                    nc.tensor.matmul(
                        out=xps[:, ts(h, 512)],
                        lhsT=ones_row,
                        rhs=xrow_b[:, ts(h, 512)],
                        start=True,
                        stop=True,
                    )
                xb = xbp.tile([128, F], BF16)
                nc.scalar.copy(out=xb, in_=xps)
                return xb

            def sample0_body(b):
                # chunk-granular first sample: starts as soon as the first
                # sigma chunk and first eps chunk land
                xb = make_xb(b)
                eb = epsp.tile([128, NCH, F], BF16, tag="eps")
                for c in range(NCH):
                    nc.gpsimd.dma_start(
                        out=eb[:, c, :], in_=epsw_t[b][:, c, :]
                    )
                t2 = t2p.tile([128, NCH, F], BF16, tag="t2")
                n_act = 7 if b % 2 == 0 else 6
                for c in range(NCH):
                    if c % 2 == 0:
                        t1h = t1p.tile([128, 2, F], BF16, tag="t1")
                    nc.vector.tensor_mul(t1h[:, c % 2, :], eb[:, c, :], sig[:, c, :])
                    nc.vector.tensor_mul(t2[:, c, :], t1h[:, c % 2, :], xb)
                    reduce_chunk(b, c, c < n_act, t2)

            def sample_body(b):
                xb = make_xb(b)

                # eps in two half-DMAs so the first TT can start sooner
                eb = epsp.tile([128, NCH, F], BF16, tag="eps")
                nc.gpsimd.dma_start(out=eb[:, :4, :], in_=epsw_t[b][:, :4, :])
                nc.gpsimd.dma_start(out=eb[:, 4:, :], in_=epsw_t[b][:, 4:, :])

                t2 = t2p.tile([128, NCH, F], BF16, tag="t2")
                xb_bc = xb[:].unsqueeze(1).broadcast_to([128, 2, F])
                n_act = 7 if b % 2 == 0 else 6
                for q in range(4):
                    qs = slice(q * 2, q * 2 + 2)
                    t1 = t1p.tile([128, 2, F], BF16, tag="t1")
                    nc.vector.tensor_mul(t1, eb[:, qs, :], sig[:, qs, :])
                    nc.vector.tensor_mul(t2[:, qs, :], t1, xb_bc)
                    for c in range(q * 2, q * 2 + 2):
                        reduce_chunk(b, c, c < n_act, t2)

            # mu/ymu setup pieces, spread one chunk per sample through the
            # early main loop so they never stall the hot pipeline
            muT = persist.tile([128, NCH, F], BF16)
            x_nat = persist.tile([BL, F], FP32)
            xT = persist.tile([128, NCH, BL], BF16)
            ymu_ps = []

            def do_mu_chunk(c):
                # muT bf16 [128p(i in k), k, o] via fp32 PE transposes
                mu_c = musp.tile([128, F], FP32, tag="mu")
                nc.sync.dma_start(out=mu_c, in_=mu_t[:, c, :])
                pt_c = ptp.tile([128, F], FP32, tag="pt")
                for k in range(NCH):
                    nc.tensor.transpose(
                        out=pt_c[:, ts(k, 128)],
                        in_=mu_c[:, ts(k, 128)],
                        identity=ident,
                    )
                # pt_c holds [i in k (part), (k, o in c)]; scatter to muT
                nc.scalar.copy(
                    out=muT[:, :, ts(c, 128)],
                    in_=pt_c[:].rearrange("p (k j) -> p k j", k=NCH),
                )

            def do_ymu():
                nc.sync.dma_start(out=x_nat, in_=x_d[:])
                for k in range(NCH):
                    ptx = ptp.tile([128, F], FP32, tag="pt")
                    nc.tensor.transpose(
                        out=ptx[:, :BL],
                        in_=x_nat[:, ts(k, 128)],
                        identity=ident[:BL, :BL],
                    )
                    nc.scalar.copy(out=xT[:, k, :], in_=ptx[:, :BL])
                # y_mu[b, o] = sum_i x[b,i] mu[o,i]  (bf16 matmul)
                for h in range(2):
                    yp = ymup.tile([BL, 512], FP32, tag=f"ymu{h}")
                    for k in range(NCH):
                        nc.tensor.matmul(
                            out=yp,
                            lhsT=xT[:, k, :],
                            rhs=muT[:, k, ts(h, 512)],
                            start=(k == 0),
                            stop=(k == NCH - 1),
                        )
                    ymu_ps.append(yp)

            mu_done = 0
            sample0_body(0)
            for b in range(1, BL):
                sample_body(b)
                if b >= 3 and mu_done < NCH:
                    do_mu_chunk(mu_done)
                    mu_done += 1
                if b == 12:
                    do_ymu()
            while mu_done < NCH:
                do_mu_chunk(mu_done)
                mu_done += 1
            if not ymu_ps:
                do_ymu()

            # -------------- C assembly + final: y = C + y2^T --------------
            C = persist.tile([BL, F], FP32)
            nc.vector.tensor_mul(C, sb_b, epsb_s)
            nc.vector.tensor_add(C, C, bmu_b)
            for h in range(2):
                nc.vector.tensor_add(C[:, ts(h, 512)], C[:, ts(h, 512)], ymu_ps[h])
            # all 8 y2 transposes into one PSUM tile, one add, one store
            pty = ptyp.tile([BL, F], FP32, tag="pty")
            for c in range(NCH):
                nc.tensor.transpose(
                    out=pty[:, ts(c, 128)], in_=y2cols[:, c, :], identity=ident
                )
            nc.vector.tensor_add(C, C, pty)
            nc.sync.dma_start(out=y_d[:], in_=C)

    nc.compile()
    return nc


_NC_CACHE: dict[int, bass.Bass] = {}


def _get_nc(BL: int) -> bass.Bass:
    if BL not in _NC_CACHE:
        _NC_CACHE[BL] = build_nc(BL)
    return _NC_CACHE[BL]


def kernel(x, weight_mu, weight_rho, bias_mu, bias_rho, eps_w, eps_b):
    B = x.shape[0]
    BL = B // N_CORES
    nc = _get_nc(BL)

    x = np.ascontiguousarray(np.asarray(x, dtype=np.float32))
    weight_mu = np.ascontiguousarray(np.asarray(weight_mu, dtype=np.float32))
    weight_rho = np.ascontiguousarray(np.asarray(weight_rho, dtype=np.float32))
    bias_mu = np.ascontiguousarray(np.asarray(bias_mu, dtype=np.float32))
    bias_rho = np.ascontiguousarray(np.asarray(bias_rho, dtype=np.float32))
    eps_w = np.ascontiguousarray(np.asarray(eps_w, dtype=np.float32))
    eps_b = np.ascontiguousarray(np.asarray(eps_b, dtype=np.float32))

    in_maps = []
    for i in range(N_CORES):
        sl = slice(i * BL, (i + 1) * BL)
        in_maps.append(
            {
                "x": x[sl],
                "weight_mu": weight_mu,
                "weight_rho": weight_rho,
                "bias_mu": bias_mu,
                "bias_rho": bias_rho,
                "eps_w": eps_w[sl],
                "eps_b": eps_b[sl],
            }
        )

    res = run_bass_kernel_spmd(nc, in_maps, core_ids=list(range(N_CORES)))
    return np.concatenate([r["y"] for r in res.results], axis=0)
